# revision 1
# baseline (speedup 1.0000x reference)
"""CompoundProteinInteractionPrediction on 8 Trainium2 NeuronCores (Bass/Tile).

v2 restructure vs baseline:
- Host pre-gathers embeddings: xs0 uploaded dim-major bf16 (replicated),
  protein word window pre-gathered per-core. No indirect DMAs on device.
- GNN row-sharded SpMM in fp8 DoubleRow perf mode (hs scaled x64 into fp8).
- Layer 0: hs0 computed from replicated xs0 (no collective).
  Layer 1/2: hs computed for own atoms only, AllGathered in fp8.
- Layer 2 needs only mean(xs3): mean(A@hs2) = colsum_A . hs2 / N. No third
  adjacency read, no SpMM. colsum partials are reduced from the fp8 slabs
  on DVE/ACT during layer 0 (hidden under the slab DMA).
- Protein CNN emitted at the collective wait points so PE fills the gaps.

Engine assignment: slab DMAs on SP/ACT queues (alternating), collective
input + readback DMAs on DVE so slab prefetch is never queued behind a
collective-blocked dma_start.
"""
import sys

sys.path.insert(0, "/opt/trn_rl_repo")

import numpy as np
import ml_dtypes

import concourse.bass as bass
import concourse.tile as tile
from concourse import bacc, mybir
from concourse.masks import make_identity

F8 = ml_dtypes.float8_e4m3
BF16 = ml_dtypes.bfloat16

DIM = 128
WINDOW = 5
KK = 2 * WINDOW + 1
LAYER_GNN = 3
LAYER_CNN = 3
LAYER_OUT = 2
HALO = WINDOW * LAYER_CNN  # 15
HS_SCALES = (1024.0, 128.0, 16.0)  # per-layer fp8 scale for hs (max ~140-165)
HS_SCALE = HS_SCALES[0]  # compat

FULL = dict(na=16384, nw=16384, nfp=100000, nword=20000, ncores=8)


def _ceil_div(a, b):
    return (a + b - 1) // b


def build_kernel(na, nw, nfp, nword, ncores, reps=1, stage=7, debug_outs=False,
                 colsum_mode="rs", dma_mode="alt", slab_bufs=8,
                 cache_pairs=8):
    local_a = na // ncores
    local_w = nw // ncores
    ach = na // 128            # 128 atom chunks of 128
    och = local_a // 128       # 16 own chunks per core
    lwin = local_w + 2 * HALO
    wch = _ceil_div(lwin, 128)
    wpad = wch * 128
    n_mb = local_a // 512      # 4 psum column tiles of 512

    f32 = mybir.dt.float32
    bf16 = mybir.dt.bfloat16
    f8 = mybir.dt.float8e4
    Relu = mybir.ActivationFunctionType.Relu
    Tanh = mybir.ActivationFunctionType.Tanh
    Ident = mybir.ActivationFunctionType.Identity
    Copy = mybir.ActivationFunctionType.Copy
    DR = mybir.MatmulPerfMode.DoubleRow
    Add = mybir.AluOpType.add
    Mult = mybir.AluOpType.mult

    nc = bacc.Bacc("TRN2", target_bir_lowering=False, debug=False,
                   enable_asserts=False, num_devices=ncores)

    # ---- DRAM inputs (per-core values via in_maps) ----
    t_bmat = nc.dram_tensor("bmat", [na, local_a], f8, kind="ExternalInput").ap()
    t_xs0 = nc.dram_tensor("xs0T", [128, na], bf16, kind="ExternalInput").ap()
    t_xs0own = nc.dram_tensor("xs0ownT", [128, local_a], bf16,
                              kind="ExternalInput").ap()
    t_img0 = nc.dram_tensor("img0", [128, wpad], bf16, kind="ExternalInput").ap()
    t_wmask = nc.dram_tensor("wmask", [128, wpad], bf16, kind="ExternalInput").ap()
    t_wgT = nc.dram_tensor("wgT", [DIM, DIM], f32, kind="ExternalInput").ap()
    t_bg = nc.dram_tensor("bg_row", [1, DIM], f32, kind="ExternalInput").ap()
    t_waT = nc.dram_tensor("waT", [DIM, DIM], bf16, kind="ExternalInput").ap()
    t_ba = nc.dram_tensor("ba_col", [DIM, 1], f32, kind="ExternalInput").ap()
    t_convm = nc.dram_tensor("convm", [DIM, KK * DIM], bf16, kind="ExternalInput").ap()
    t_convb = nc.dram_tensor("convb_col", [DIM, 1], f32, kind="ExternalInput").ap()
    t_woT = nc.dram_tensor("woT", [2 * DIM, 2 * DIM], f32, kind="ExternalInput").ap()
    t_bo = nc.dram_tensor("bo_col", [2 * DIM, 1], f32, kind="ExternalInput").ap()
    t_wiT = nc.dram_tensor("wiT", [2 * DIM, 2], f32, kind="ExternalInput").ap()
    t_bi = nc.dram_tensor("bi_col", [2, 1], f32, kind="ExternalInput").ap()
    t_ones = nc.dram_tensor("ones_row", [1, DIM], bf16, kind="ExternalInput").ap()
    t_csg = nc.dram_tensor("csg_own", [DIM, 16], f32, kind="ExternalInput").ap()
    t_out = nc.dram_tensor("out", [2, 1], f32, kind="ExternalOutput").ap()
    if debug_outs:
        t_dxs1 = nc.dram_tensor("d_xs1", [128, local_a], f32, kind="ExternalOutput").ap()
        t_dxs2 = nc.dram_tensor("d_xs2", [128, local_a], f32, kind="ExternalOutput").ap()
        t_dcs = nc.dram_tensor("d_cs", [128, na // 128], f32, kind="ExternalOutput").ap()
        t_dhs1 = nc.dram_tensor("d_hs1", [128, 2048], f32, kind="ExternalOutput").ap()
        t_dcomp = nc.dram_tensor("d_comp", [128, 1], f32, kind="ExternalOutput").ap()
        t_ddot = nc.dram_tensor("d_dot", [128, 1], f32, kind="ExternalOutput").ap()
        t_dimg = nc.dram_tensor("d_img", [128, 512], f32, kind="ExternalOutput").ap()
        t_dhsp = nc.dram_tensor("d_hsp", [128, 512], f32, kind="ExternalOutput").ap()
        t_dys = nc.dram_tensor("d_ys", [128, 1], f32, kind="ExternalOutput").ap()

    rg = [list(range(ncores))]
    bmat_r = t_bmat.rearrange("(t p) m -> t p m", p=128)

    with tile.TileContext(nc) as tc:
        with (
            tc.tile_pool(name="persist", bufs=1) as persist,
            tc.tile_pool(name="hsp", bufs=4) as hsp,
            tc.tile_pool(name="slabp", bufs=slab_bufs) as slabp,
            tc.tile_pool(name="xsp", bufs=2) as xsp,
            tc.tile_pool(name="convp", bufs=3) as convp,
            tc.tile_pool(name="hsfp", bufs=1) as hsfp,
            tc.tile_pool(name="scrp", bufs=1) as scrp,
            tc.tile_pool(name="stp", bufs=4) as stp,
            tc.tile_pool(name="smallp", bufs=12) as smallp,
            tc.tile_pool(name="catp", bufs=3) as catp,
            tc.tile_pool(name="dram", bufs=1, space="DRAM") as dram,
            tc.tile_pool(name="ps_spmm", bufs=1, space="PSUM") as ps_spmm,
            tc.tile_pool(name="ps_misc", bufs=2, space="PSUM") as ps_misc,
            tc.tile_pool(name="ps_conv", bufs=2, space="PSUM") as ps_conv,
        ):
            # ---- persistent small weights ----
            wgT = persist.tile([DIM, DIM], f32, tag="wgT")
            wgT_bf = persist.tile([DIM, DIM], bf16, tag="wgT_bf")
            bg_row = persist.tile([1, DIM], f32, tag="bg_row")
            waT = persist.tile([DIM, DIM], bf16, tag="waT")
            ba_col = persist.tile([DIM, 1], f32, tag="ba_col")
            convm = persist.tile([DIM, KK * DIM], bf16, tag="convm")
            convb_col = persist.tile([DIM, 1], f32, tag="convb_col")
            woT_sb = persist.tile([DIM, 4 * DIM], f32, tag="woT_sb")
            bo_sb = persist.tile([DIM, 2], f32, tag="bo_sb")
            wiT_sb = persist.tile([DIM, 4], f32, tag="wiT_sb")
            bi_sb = persist.tile([2, 1], f32, tag="bi_sb")
            ones_row = persist.tile([1, DIM], bf16, tag="ones_row")
            ones_f32 = persist.tile([1, DIM], f32, tag="ones_f32")
            ones_col1 = persist.tile([1, 1], bf16, tag="ones_col1")
            bg_bf = persist.tile([1, DIM], bf16, tag="bg_bf")
            xs0_full = persist.tile([128, na], bf16, tag="xs0_full")
            xs0_own = persist.tile([128, local_a], bf16, tag="xs0_own")
            img0 = persist.tile([128, wpad], bf16, tag="img0")
            wmask_sb = persist.tile([128, wpad], bf16, tag="wmask_sb")
            cs_sb = persist.tile([128, ach], f32, tag="cs_sb")       # colsum partials
            cs_bf = persist.tile([128, ach], bf16, tag="cs_bf")
            csg_bf = persist.tile([DIM, och], bf16, tag="csg_bf")    # own-k global colsum
            ident = persist.tile([DIM, DIM], f32, tag="ident")
            bcache = (persist.tile([128, cache_pairs * 2 * local_a], f8,
                                   tag="bcache", name="bcache") if cache_pairs else None)
            hs_pT = persist.tile([128, local_w], bf16, tag="hspT")
            w_bf = persist.tile([1, local_w], bf16, tag="wbf")
            comp_sum = persist.tile([128, 1], f32, tag="csum")
            ys_ar = persist.tile([128, 1], f32, tag="ysar")

            nc.sync.dma_start(wgT[:], t_wgT[:])
            nc.vector.tensor_copy(wgT_bf[:], wgT[:])
            nc.sync.dma_start(bg_row[:], t_bg[:])
            nc.sync.dma_start(waT[:], t_waT[:])
            nc.sync.dma_start(ba_col[:], t_ba[:])
            nc.sync.dma_start(convm[:], t_convm[:])
            nc.sync.dma_start(convb_col[:], t_convb[:])
            for j in range(2):
                for i in range(2):
                    nc.sync.dma_start(
                        woT_sb[:, (j * 2 + i) * DIM:(j * 2 + i + 1) * DIM],
                        t_woT[j * DIM:(j + 1) * DIM, i * DIM:(i + 1) * DIM])
                nc.sync.dma_start(bo_sb[:, j:j + 1], t_bo[j * DIM:(j + 1) * DIM, :])
                nc.sync.dma_start(wiT_sb[:, 2 * j:2 * j + 2], t_wiT[j * DIM:(j + 1) * DIM, :])
            nc.sync.dma_start(bi_sb[:], t_bi[:])
            nc.sync.dma_start(ones_row[:], t_ones[:])
            nc.sync.dma_start(ones_col1[:], t_ones[:, 0:1])
            nc.gpsimd.memset(ones_f32[:], 1.0)
            make_identity(nc, ident[:])
            nc.vector.tensor_copy(bg_bf[:], bg_row[:])
            nc.sync.dma_start(xs0_full[:], t_xs0[:])
            nc.sync.dma_start(xs0_own[:], t_xs0own[:])
            nc.sync.dma_start(img0[:], t_img0[:])
            if colsum_mode == "host":
                csg_f32 = smallp.tile([DIM, 16], f32, tag="csgf")
                nc.sync.dma_start(csg_f32[:], t_csg[:])
                nc.vector.tensor_copy(csg_bf[:], csg_f32[:])
            nc.sync.dma_start(wmask_sb[:], t_wmask[:])

            def spmm_layer(layer, rep, hs_src, do_colsum):
                """SpMM over own rows via fp8 DoubleRow; psum [dims, local_a].
                hs_src(kj) -> lhsT AP [128, 2, DIM] f8 for chunk pair kj."""
                psums = [ps_spmm.tile([128, 512], f32, tag=f"spmm{mb}",
                                      name=f"spmm_r{rep}_l{layer}_{mb}")
                         for mb in range(n_mb)]
                kc0 = ach // 2 - cache_pairs
                for kj in range(ach // 2):
                    if cache_pairs and kj >= kc0:
                        slab = bcache[:, (kj - kc0) * 2 * local_a:
                                      (kj - kc0 + 1) * 2 * local_a]
                        if layer == 0:
                            eng = nc.sync if kj % 2 == 0 else nc.scalar
                            eng.dma_start(slab[:, :local_a], bmat_r[2 * kj])
                            eng.dma_start(slab[:, local_a:], bmat_r[2 * kj + 1])
                        slab3 = slab.rearrange("p (two m) -> p two m", two=2)
                        lhsT = hs_src(kj)
                        for mb in range(n_mb):
                            nc.tensor.matmul(
                                psums[mb][:], lhsT,
                                slab3[:, :, mb * 512:(mb + 1) * 512],
                                start=(kj == 0), stop=(kj == ach // 2 - 1),
                                perf_mode=DR)
                        if do_colsum:
                            nc.vector.reduce_sum(
                                cs_sb[:, 2 * kj:2 * kj + 1], slab[:, :local_a],
                                axis=mybir.AxisListType.X)
                            scr = scrp.tile([128, local_a], f8, tag="csscr")
                            nc.scalar.activation(
                                scr[:], slab[:, local_a:], Copy,
                                accum_out=cs_sb[:, 2 * kj + 1:2 * kj + 2])
                        continue
                    slab = slabp.tile([128, 2 * local_a], f8, tag="slab")
                    if dma_mode == "sp":
                        nc.sync.dma_start(slab[:, :local_a], bmat_r[2 * kj])
                        nc.sync.dma_start(slab[:, local_a:], bmat_r[2 * kj + 1])
                    elif dma_mode == "alt":
                        eng = nc.sync if kj % 2 == 0 else nc.scalar
                        eng.dma_start(slab[:, :local_a], bmat_r[2 * kj])
                        eng.dma_start(slab[:, local_a:], bmat_r[2 * kj + 1])
                    elif dma_mode == "alt3":
                        eng = (nc.sync, nc.scalar, nc.gpsimd)[kj % 3]
                        eng.dma_start(slab[:, :local_a], bmat_r[2 * kj])
                        eng.dma_start(slab[:, local_a:], bmat_r[2 * kj + 1])
                    else:  # "half": one half per engine every pair
                        nc.sync.dma_start(slab[:, :local_a], bmat_r[2 * kj])
                        nc.scalar.dma_start(slab[:, local_a:], bmat_r[2 * kj + 1])
                    slab3 = slab[:].rearrange("p (two m) -> p two m", two=2)
                    lhsT = hs_src(kj)
                    for mb in range(n_mb):
                        nc.tensor.matmul(
                            psums[mb][:], lhsT, slab3[:, :, mb * 512:(mb + 1) * 512],
                            start=(kj == 0), stop=(kj == ach // 2 - 1),
                            perf_mode=DR)
                    if do_colsum:
                        # colsum partials; DVE / ACT halves (layer 0 only)
                        nc.vector.reduce_sum(
                            cs_sb[:, 2 * kj:2 * kj + 1], slab[:, :local_a],
                            axis=mybir.AxisListType.X)
                        scr = scrp.tile([128, local_a], f8, tag="csscr")
                        nc.scalar.activation(
                            scr[:], slab[:, local_a:], Copy,
                            accum_out=cs_sb[:, 2 * kj + 1:2 * kj + 2])
                return psums

            def hs0_pair(kj):
                """hs0 for chunk pair kj, on the fly from replicated xs0."""
                hp = ps_misc.tile([128, 256], f32, tag="m256")
                hch = hsp.tile([128, 256], f8, tag="hs0c")
                for i in range(2):
                    ki = 2 * kj + i
                    nc.tensor.matmul(hp[:, i * 128:(i + 1) * 128],
                                     ones_row[:], bg_bf[:], start=True, stop=False)
                    nc.tensor.matmul(hp[:, i * 128:(i + 1) * 128],
                                     xs0_full[:, ki * 128:(ki + 1) * 128],
                                     wgT_bf[:], start=False, stop=True)
                nc.scalar.activation(hch[:], hp[:], Relu, scale=HS_SCALES[0])
                return hch[:].rearrange("p (two d) -> p two d", two=2)

            def own_hs(xs_c, rep, layer):
                """hs (fp8, x64) for own atoms from xs_c [128(dim), local_a] f32."""
                hso = hsp.tile([128, och * 128], f8, tag="hso",
                               name=f"hso_r{rep}_l{layer}")
                for t in range(och):
                    hp = ps_misc.tile([128, 256], f32, tag="m256")
                    nc.tensor.matmul(hp[:, :128], ones_f32[:], bg_row[:],
                                     start=True, stop=False)
                    nc.tensor.matmul(hp[:, :128], xs_c[:, t * 128:(t + 1) * 128],
                                     wgT[:], start=False, stop=True)
                    nc.scalar.activation(hso[:, t * 128:(t + 1) * 128], hp[:, :128],
                                         Relu, scale=HS_SCALES[layer])
                return hso

            def allgather_hs(hso, rep, layer):
                """own hs f8 [128, 2048] -> hs_full f8 [128, ach*128].
                All DMAs on DVE queue (keeps SP/ACT slab prefetch unblocked)."""
                agi = dram.tile([128, och * 128], f8, tag=f"agi_r{rep}_l{layer}",
                                name=f"agi_r{rep}_l{layer}")
                ago = dram.tile([128 * ncores, och * 128], f8,
                                tag=f"ago_r{rep}_l{layer}",
                                name=f"ago_r{rep}_l{layer}", addr_space="Shared")
                nc.gpsimd.dma_start(agi[:], hso[:])
                nc.gpsimd.collective_compute(
                    "AllGather", mybir.AluOpType.bypass,
                    ins=[agi[:].opt()], outs=[ago[:].opt()], replica_groups=rg)
                hs_full = hsfp.tile([128, ach * 128], f8, tag="hsfull",
                                    name=f"hsfull_r{rep}_l{layer}")
                for c in range(ncores):
                    nc.gpsimd.dma_start(
                        hs_full[:, c * och * 128:(c + 1) * och * 128],
                        ago[c * 128:(c + 1) * 128, :])
                return hs_full

            def xs_update(psums, base_bf, rep, layer):
                """xs_new f32 = base + psum/HS_SCALES[layer]."""
                xs_c = xsp.tile([128, local_a], f32, tag="xs_c",
                                name=f"xs_r{rep}_l{layer}")
                for mb in range(n_mb):
                    nc.vector.scalar_tensor_tensor(
                        xs_c[:, mb * 512:(mb + 1) * 512], psums[mb][:],
                        1.0 / HS_SCALES[layer], base_bf[:, mb * 512:(mb + 1) * 512],
                        op0=Mult, op1=Add)
                return xs_c

            # ---------------- protein conv pieces ----------------
            def conv_layer(src, l, rep):
                lo = WINDOW * (l + 1)
                hi = lwin - WINDOW * (l + 1)
                dst = convp.tile([128, wpad], bf16, tag="convb",
                                 name=f"conv_r{rep}_l{l}")
                o = lo
                while o < hi:
                    ms = min(512, hi - o)
                    pc = ps_conv.tile([128, 512], f32, tag="pconv")
                    for a in range(KK):
                        nc.tensor.matmul(
                            pc[:, :ms], convm[:, a * DIM:(a + 1) * DIM],
                            src[:, o + a - WINDOW:o + a - WINDOW + ms],
                            start=(a == 0), stop=(a == KK - 1))
                    nc.scalar.activation(dst[:, o:o + ms], pc[:, :ms], Relu,
                                         bias=convb_col[:])
                    o += ms
                if l < LAYER_CNN - 1:
                    nc.vector.tensor_tensor(dst[:, lo:hi], dst[:, lo:hi],
                                            wmask_sb[:, lo:hi], op=Mult)
                return dst

            def protein_hsp(xsp_img):
                o = 0
                while o < local_w:
                    ms = min(512, local_w - o)
                    pc = ps_conv.tile([128, 512], f32, tag="pconv")
                    nc.tensor.matmul(pc[:, :ms], waT[:],
                                     xsp_img[:, HALO + o:HALO + o + ms],
                                     start=True, stop=True)
                    nc.scalar.activation(hs_pT[:, o:o + ms], pc[:, :ms], Relu,
                                         bias=ba_col[:])
                    o += ms

            def tail(rep):
                # h = relu(Wa @ (comp_sum/na) + ba)
                comp_bf = smallp.tile([128, 1], bf16, tag="smallbf")
                nc.vector.tensor_scalar_mul(comp_bf[:], comp_sum[:], 1.0 / na)
                ph = ps_conv.tile([128, 512], f32, tag="pconv")
                nc.tensor.matmul(ph[:, :1], waT[:], comp_bf[:], start=True, stop=True)
                h_bf = smallp.tile([128, 1], bf16, tag="smallbf")
                nc.scalar.activation(h_bf[:], ph[:, :1], Relu, bias=ba_col[:])
                # w = tanh(h . hs_p)
                o = 0
                while o < local_w:
                    ms = min(512, local_w - o)
                    pw = ps_conv.tile([128, 512], f32, tag="pconv")
                    nc.tensor.matmul(pw[:1, :ms], h_bf[:], hs_pT[:, o:o + ms],
                                     start=True, stop=True)
                    nc.scalar.activation(w_bf[:, o:o + ms], pw[:1, :ms], Tanh)
                    o += ms
                # ys partial = sum_l w[l] * hs_p[:, l]
                prev = None
                o = 0
                while o < local_w:
                    ms = min(512, local_w - o)
                    pb = ps_conv.tile([128, 512], f32, tag="pconv")
                    nc.tensor.matmul(pb[:, :ms], ones_row[:], w_bf[:, o:o + ms],
                                     start=True, stop=True)
                    scr = stp.tile([128, 512], f32, tag="ysscr")
                    nc.vector.tensor_tensor(scr[:, :ms], pb[:, :ms],
                                            hs_pT[:, o:o + ms], op=Mult)
                    acc = smallp.tile([128, 1], f32, tag="small")
                    nc.vector.reduce_sum(acc[:], scr[:, :ms],
                                         axis=mybir.AxisListType.X)
                    if prev is not None:
                        nc.vector.tensor_tensor(acc[:], acc[:], prev[:], op=Add)
                    prev = acc
                    o += ms
                arp_i = dram.tile([128, 1], f32, tag=f"arpi_r{rep}",
                                  name=f"arpi_r{rep}")
                arp_o = dram.tile([128, 1], f32, tag=f"arpo_r{rep}",
                                  name=f"arpo_r{rep}", addr_space="Shared")
                nc.gpsimd.dma_start(arp_i[:], prev[:])
                nc.gpsimd.collective_compute(
                    "AllReduce", Add, ins=[arp_i[:].opt()], outs=[arp_o[:].opt()],
                    replica_groups=rg)
                nc.gpsimd.dma_start(ys_ar[:], arp_o[:])
                if debug_outs and rep == 0:
                    nc.sync.dma_start(t_dys[:], ys_ar[:])
                # output MLP on cat = [compound, protein]
                cat = catp.tile([128, 2], f32, tag="cat")
                nc.vector.tensor_scalar_mul(cat[:, 0:1], comp_sum[:], 1.0 / na)
                nc.vector.tensor_scalar_mul(cat[:, 1:2], ys_ar[:], 1.0 / nw)
                for l in range(LAYER_OUT):
                    ncat = catp.tile([128, 2], f32, tag="cat")
                    for i in range(2):
                        pm = ps_misc.tile([128, 256], f32, tag="m256")
                        for j in range(2):
                            nc.tensor.matmul(
                                pm[:, :1],
                                woT_sb[:, (j * 2 + i) * DIM:(j * 2 + i + 1) * DIM],
                                cat[:, j:j + 1], start=(j == 0), stop=(j == 1))
                        nc.scalar.activation(ncat[:, i:i + 1], pm[:, :1], Relu,
                                             bias=bo_sb[:, i:i + 1])
                    cat = ncat
                pf = ps_misc.tile([128, 256], f32, tag="m256")
                for j in range(2):
                    nc.tensor.matmul(pf[:2, :1], wiT_sb[:, 2 * j:2 * j + 2],
                                     cat[:, j:j + 1], start=(j == 0), stop=(j == 1))
                res = smallp.tile([2, 1], f32, tag="res")
                nc.scalar.activation(res[:], pf[:2, :1], Ident, bias=bi_sb[:])
                nc.sync.dma_start(t_out[:], res[:])

            def finish_early():
                res2 = smallp.tile([2, 1], f32, tag="res")
                nc.vector.tensor_copy(res2[:], bi_sb[:])
                nc.sync.dma_start(t_out[:], res2[:])

            # ================= main body (reps > 1 for timing) =================
            for rep in range(reps):
                if stage < 2:
                    break
                # ---- GNN layer 0 (+ colsum partials) ----
                psums0 = spmm_layer(0, rep, hs0_pair,
                                    do_colsum=(colsum_mode == "rs"))
                xs1_c = xs_update(psums0, xs0_own, rep, 0)
                if debug_outs and rep == 0:
                    nc.sync.dma_start(t_dxs1[:], xs1_c[:])
                    nc.sync.dma_start(t_dcs[:], cs_sb[:])
                if stage < 3:
                    continue
                # ---- own hs1; conv layer 1 fills the AllGather gap ----
                hso1 = own_hs(xs1_c, rep, 1)
                if debug_outs and rep == 0:
                    dh1 = stp.tile([128, 512], f32, tag="ysscr")
                    for _t in range(4):
                        nc.scalar.activation(dh1[:, _t * 128:(_t + 1) * 128],
                                             hso1[:, _t * 128:(_t + 1) * 128], Ident)
                    nc.sync.dma_start(t_dhs1[:, :512], dh1[:])
                imgb = conv_layer(img0, 0, rep)
                if stage < 4:
                    continue
                hs1_full = allgather_hs(hso1, rep, 1)
                if colsum_mode == "rs":
                    # ReduceScatter csPartial -> own-k global colsum; issued
                    # behind AG1 so it completes during the layer-1 SpMM
                    pt = ps_misc.tile([128, 256], f32, tag="m256")
                    nc.tensor.transpose(pt[:, :128], cs_sb[:], ident[:])
                    cs_t = smallp.tile([128, 128], f32, tag="cst")
                    nc.vector.tensor_copy(cs_t[:], pt[:, :128])
                    rs_i = dram.tile([128, 128], f32, tag=f"rsi_r{rep}",
                                     name=f"rsi_r{rep}")
                    rs_o = dram.tile([16, 128], f32, tag=f"rso_r{rep}",
                                     name=f"rso_r{rep}")
                    nc.gpsimd.dma_start(rs_i[:], cs_t[:])
                    nc.gpsimd.collective_compute(
                        "ReduceScatter", Add, ins=[rs_i[:].opt()],
                        outs=[rs_o[:].opt()], replica_groups=rg)
                    csg_sb = smallp.tile([16, 128], f32, tag="csg16")
                    nc.gpsimd.dma_start(csg_sb[:], rs_o[:])
                    ptb = ps_misc.tile([128, 256], f32, tag="m256")
                    nc.tensor.transpose(ptb[:, :16], csg_sb[:], ident[:16, :16])
                    nc.vector.tensor_copy(csg_bf[:], ptb[:, :16])
                # ---- GNN layer 1 ----
                hs1_r = hs1_full[:].rearrange("p (t d) -> p t d", d=128)
                psums1 = spmm_layer(1, rep,
                                    lambda kj: hs1_r[:, 2 * kj:2 * kj + 2, :],
                                    do_colsum=False)
                xs2_c = xs_update(psums1, xs1_c, rep, 1)
                if debug_outs and rep == 0:
                    nc.sync.dma_start(t_dxs2[:], xs2_c[:])
                if stage < 5:
                    continue
                # ---- layer 2: only mean(xs3) is needed ----
                hso2 = own_hs(xs2_c, rep, 2)
                rsum = smallp.tile([128, 1], f32, tag="small")
                nc.vector.reduce_sum(rsum[:], xs2_c[:], axis=mybir.AxisListType.X)
                # dot over OWN chunks: sum_{k own} csg[k] * hs2_own[k, :]
                pdot = ps_conv.tile([128, 512], f32, tag="pconv")
                for t in range(och):
                    nc.tensor.matmul(pdot[:1, :128], csg_bf[:, t:t + 1],
                                     hso2[:, t * 128:(t + 1) * 128],
                                     start=(t == 0), stop=(t == och - 1))
                dot_sb = smallp.tile([1, 128], bf16, tag="dotsb")
                nc.scalar.activation(dot_sb[:], pdot[:1, :128], Ident)
                # transpose [1,128] -> [128,1] via K=1 matmul against ones
                pdt = ps_misc.tile([128, 256], f32, tag="m256")
                nc.tensor.matmul(pdt[:, :1], dot_sb[:], ones_col1[:],
                                 start=True, stop=True)
                # comp partial = rowsum(xs2_c) + dot/HS_SCALE
                cpart = smallp.tile([128, 1], f32, tag="small")
                nc.vector.scalar_tensor_tensor(cpart[:], pdt[:, :1],
                                               1.0 / HS_SCALES[2], rsum[:],
                                               op0=Mult, op1=Add)
                arc_i = dram.tile([128, 1], f32, tag=f"arci_r{rep}",
                                  name=f"arci_r{rep}")
                arc_o = dram.tile([128, 1], f32, tag=f"arco_r{rep}",
                                  name=f"arco_r{rep}", addr_space="Shared")
                nc.gpsimd.dma_start(arc_i[:], cpart[:])
                nc.gpsimd.collective_compute(
                    "AllReduce", Add, ins=[arc_i[:].opt()], outs=[arc_o[:].opt()],
                    replica_groups=rg)
                # conv layers 2,3 + protein attention fill the AllReduce gap
                imgc = conv_layer(imgb, 1, rep)
                imgd = conv_layer(imgc, 2, rep)
                protein_hsp(imgd)
                nc.gpsimd.dma_start(comp_sum[:], arc_o[:])
                if debug_outs and rep == 0:
                    nc.sync.dma_start(t_dcomp[:], comp_sum[:])
                    nc.sync.dma_start(t_ddot[:], cpart[:])
                    dimg = stp.tile([128, 512], f32, tag="ysscr")
                    nc.scalar.activation(dimg[:], imgd[:, HALO:HALO + 512], Ident)
                    nc.sync.dma_start(t_dimg[:], dimg[:])
                    dhsp = stp.tile([128, 512], f32, tag="ysscr")
                    nc.scalar.activation(dhsp[:], hs_pT[:, :512], Ident)
                    nc.sync.dma_start(t_dhsp[:], dhsp[:])
                if stage < 7:
                    continue
                tail(rep)

            if stage < 7:
                finish_early()

    nc.compile()
    return nc


def prep_in_maps(inputs, na, nw, nfp, nword, ncores):
    """Host-side sharding/layout prep (gathers + casts + transposes)."""
    local_a = na // ncores
    local_w = nw // ncores
    lwin = local_w + 2 * HALO
    wch = _ceil_div(lwin, 128)
    wpad = wch * 128

    fingerprints = np.asarray(inputs["fingerprints"]).astype(np.int64)
    adjacency = np.asarray(inputs["adjacency"], dtype=np.float32)
    words = np.asarray(inputs["words"]).astype(np.int64)
    embed_fp = np.asarray(inputs["embed_fp"], dtype=np.float32)
    embed_word = np.asarray(inputs["embed_word"], dtype=np.float32)
    Wg = np.asarray(inputs["Wg"], dtype=np.float32)
    bg = np.asarray(inputs["bg"], dtype=np.float32)
    conv_k = np.asarray(inputs["conv_k"], dtype=np.float32)
    conv_b = np.asarray(inputs["conv_b"], dtype=np.float32)
    Wa = np.asarray(inputs["Wa"], dtype=np.float32)
    ba = np.asarray(inputs["ba"], dtype=np.float32)
    Wo = np.asarray(inputs["Wo"], dtype=np.float32)
    bo = np.asarray(inputs["bo"], dtype=np.float32)
    Wi = np.asarray(inputs["Wi"], dtype=np.float32)
    bi = np.asarray(inputs["bi"], dtype=np.float32)

    # adjacency -> fp8 bit pattern (0.0 -> 0x00, nonzero -> 0x38 = 1.0 in e4m3)
    nz = adjacency != 0
    a8 = nz.astype(np.uint8) * np.uint8(0x38)
    colsum_g = nz.sum(axis=0).astype(np.float32)               # [na]

    # xs0 gather, dim-major, bf16 (replicated)
    xs0 = embed_fp[fingerprints]                       # [na, DIM] f32
    xs0T = np.ascontiguousarray(xs0.T).astype(BF16)    # [128, na]

    K2 = conv_k[0, 0]
    M = np.zeros((DIM, KK * DIM), np.float32)
    for a in range(KK):
        Ma = np.zeros((DIM, DIM), np.float32)
        for b_ in range(KK):
            Ma += K2[a, b_] * np.eye(DIM, k=5 - b_, dtype=np.float32)
        M[:, a * DIM:(a + 1) * DIM] = Ma

    common = dict(
        xs0T=xs0T,
        wgT=np.ascontiguousarray(Wg.T).astype(np.float32),
        bg_row=bg.reshape(1, DIM).astype(np.float32),
        waT=np.ascontiguousarray(Wa.T).astype(BF16),
        ba_col=ba.reshape(DIM, 1).astype(np.float32),
        convm=M.astype(BF16),
        convb_col=np.full((DIM, 1), conv_b[0], np.float32),
        woT=np.ascontiguousarray(Wo.T).astype(np.float32),
        bo_col=bo.reshape(2 * DIM, 1).astype(np.float32),
        wiT=np.ascontiguousarray(Wi.T).astype(np.float32),
        bi_col=bi.reshape(2, 1).astype(np.float32),
        ones_row=np.ones((1, DIM), BF16),
    )

    ws_full = embed_word[words]                        # [nw, DIM] f32

    in_maps = []
    for c in range(ncores):
        sl = slice(c * local_a, (c + 1) * local_a)
        bmat = np.ascontiguousarray(a8[sl, :].T).view(F8)
        w0 = c * local_w - HALO
        pos = np.arange(wpad)
        gidx = w0 + pos
        valid = (gidx >= 0) & (gidx < nw) & (pos < lwin)
        win = np.where(valid[:, None], ws_full[np.clip(gidx, 0, nw - 1)], 0.0)
        img0 = np.ascontiguousarray(win.T).astype(BF16)   # [128, wpad]
        wmask = np.broadcast_to(
            ((gidx >= 0) & (gidx < nw)).astype(BF16)[None, :], (128, wpad))
        m = dict(common)
        m.update(bmat=bmat, img0=img0,
                 wmask=np.ascontiguousarray(wmask),
                 xs0ownT=np.ascontiguousarray(xs0T[:, sl]),
                 csg_own=np.ascontiguousarray(
                     colsum_g[sl].reshape(16, 128).T))
        in_maps.append(m)
    return in_maps


_CACHE = {}


def _get_kernel(cfg_key):
    if cfg_key not in _CACHE:
        na, nw, nfp, nword, ncores = cfg_key
        _CACHE[cfg_key] = build_kernel(na, nw, nfp, nword, ncores)
    return _CACHE[cfg_key]


def kernel(**inputs) -> np.ndarray:
    from concourse import bass_utils
    cfg = FULL
    key = (cfg["na"], cfg["nw"], cfg["nfp"], cfg["nword"], cfg["ncores"])
    nc = _get_kernel(key)
    in_maps = prep_in_maps(inputs, *key)
    res = bass_utils.run_bass_kernel_spmd(
        nc, in_maps, core_ids=list(range(cfg["ncores"])), trace=False)
    out = np.asarray(res.results[0]["out"], np.float32).reshape(1, 2)
    return out

